# revision 21
# baseline (speedup 1.0000x reference)
"""Trainium2 Bass kernel for nn_C4StandardTransformer (MoE-routed transformer step).

kernel(**inputs) takes the FULL inputs (state [32768,16] + expert weights),
shards the batch across 8 NeuronCores (pure data parallel), runs an on-device
MoE-routed Bass kernel per core, and returns the full [32768,16] output.

Key algorithmic facts exploited:
 - The reference's attention softmax is over a length-1 axis, so w == 1 and
   Q/K/Wq/Wk are dead; attn = xn @ (Wo[e] @ Wv[e]).T.
 - The opcode slot holds exact integers, so the soft top-hat gates take only
   the constant values g0 = sigmoid(10)^2 (own expert), g1 ~ 4.54e-5
   (neighbors), g2 ~ 9.4e-14 (negligible). The kernel computes the top-1
   (own-expert) term exactly and optionally the two neighbor terms (TOPK=3).
 - Tokens are routed on device: counting-sort ranks are computed with DVE
   one-hot/prefix tricks plus one PE matmul against a strict-triangular
   constant, then tokens are scattered into a rank-major sorted buffer via
   indirect DMA, processed per 8-expert supergroup in an 8-token-stacked
   [128, 160] layout with block-diagonal fp16 matmuls, and gathered back.
"""
import sys
import numpy as np

for _p in ("/opt/trn_rl_repo", "/root/.axon_site/_ro/trn_rl_repo"):
    if _p not in sys.path:
        sys.path.append(_p)

TOPK = 1




E, D, DFF, OPCODE, EPS = 39, 16, 64, 6, 1e-5
Bc = 4096            # tokens per core
P = 128              # partitions
NCOL = Bc // P       # 32 free-dim token slots per partition
PADSZ = 160          # slots per expert per core
NE = 40              # padded expert count (8*5)
NSG = 5              # supergroups
NROW = PADSZ * NE    # sorted buffer rows
G0 = float(1.0 / (1.0 + np.exp(-10.0))) ** 2
G1 = float((1.0 / (1.0 + np.exp(-30.0))) * (1.0 / (1.0 + np.exp(10.0))))


def prep_consts(Wq, Wk, Wv, Wo, W1, b1, W2, b2, topk=1):
    """Host-side constant packing. Returns dict name -> np.ndarray.
    For topk=3, weight sets for expert shifts j in (-1, 0, +1) are packed
    (shift j means slot expert e is processed with weights of e+j; out-of
    range -> zeros)."""
    Wov = np.einsum('ejv,evd->ejd', Wo, Wv).astype(np.float32)  # attn = Wov @ xn
    # fold LayerNorm mean-subtraction into the contraction weights
    Wov = Wov - Wov.mean(axis=2, keepdims=True)
    W1 = (W1 - W1.mean(axis=2, keepdims=True)).astype(np.float32)

    def wslot(Warr, e, fill_shape):
        if 0 <= e < E:
            return Warr[e]
        return np.zeros(fill_shape, Warr.dtype)

    shifts = [0] if topk == 1 else [-1, 0, 1]
    consts = {}
    # [1, 39] expert iota and [1, 32, 32] strict-lower mask, [128,128] helpers
    consts["c_iota"] = np.arange(E, dtype=np.float32).reshape(1, 1, E)
    lt = np.tril(np.ones((NCOL, NCOL), np.float32), -1)  # mask[n, n'] = n' < n
    consts["c_ltmask"] = lt.reshape(1, NCOL, NCOL)
    # C1 matmul stationary: lhsT[k, m] = 1 if k < m (strict upper)
    consts["c_uones"] = np.triu(np.ones((P, P), np.float32), 1)
    consts["c_id32"] = np.eye(P, dtype=np.float32)
    consts["c_id16"] = np.eye(P, dtype=np.float16)
    # block-ones for stacked mean: out[t*16+j, c] = sum_d rhs[t*16+d, c]/16
    onesbd = np.zeros((P, P), np.float32)
    for t in range(8):
        onesbd[t*16:(t+1)*16, t*16:(t+1)*16] = 1.0 / 16.0
    consts["c_onesbd"] = onesbd

    for j in shifts:
        tag = {0: "", -1: "m", 1: "p"}[j]
        wA = np.zeros((NSG, P, P), np.float16)
        wB = np.zeros((NSG, 4, P, P), np.float16)
        b1s = np.zeros((NSG, 4, P, 1), np.float32)
        wC = np.zeros((NSG, 4, P, 32), np.float16)
        b2s = np.zeros((NSG, P, 1), np.float32)
        for s in range(NSG):
            for t in range(8):
                e = 8 * s + t + j
                wv = wslot(Wov, e, (D, D))
                # attn[d_out] = sum_d wv[d_out, d] xn[d]; lhsT[k=d, m=d_out]
                wA[s, t*16:(t+1)*16, t*16:(t+1)*16] = wv.T.astype(np.float16)
                b2s[s, t*16:(t+1)*16, 0] = wslot(b2, e, (D,))
            for i in range(4):
                for tt in range(2):
                    e = 8 * s + 2 * i + tt + j
                    w1 = wslot(W1, e, (DFF, D))     # h[f] = sum_d w1[f,d] xn2[d]
                    t = 2 * i + tt                   # input slot
                    wB[s, i, t*16:(t+1)*16, tt*64:(tt+1)*64] = w1.T.astype(np.float16)
                    b1s[s, i, tt*64:(tt+1)*64, 0] = wslot(b1, e, (DFF,))
                    w2 = wslot(W2, e, (D, DFF))     # ffn[d] = sum_f w2[d,f] h[f]
                    wC[s, i, tt*64:(tt+1)*64, tt*16:(tt+1)*16] = w2.T.astype(np.float16)
        consts[f"c_wA{tag}"] = np.ascontiguousarray(wA.transpose(1, 0, 2))
        consts[f"c_wB{tag}"] = np.ascontiguousarray(wB.transpose(2, 0, 1, 3))
        consts[f"c_b1s{tag}"] = np.ascontiguousarray(b1s.transpose(2, 0, 1, 3))
        consts[f"c_wC{tag}"] = np.ascontiguousarray(wC.transpose(2, 0, 1, 3))
        consts[f"c_b2s{tag}"] = np.ascontiguousarray(b2s.transpose(1, 0, 2))
    return consts


def build_kernel(topk=1, stop_after=None, debug=False):
    """Build the Bacc module. stop_after in (None, 'route', 'scatter',
    'compute') truncates for debugging (intermediates stay in dram tensors)."""
    import concourse.bass as bass
    import concourse.bacc as bacc
    import concourse.tile as tile
    from concourse import mybir
    from concourse.bass import IndirectOffsetOnAxis

    f32, f16, i32 = mybir.dt.float32, mybir.dt.float16, mybir.dt.int32
    AX = mybir.AxisListType.X
    OP = mybir.AluOpType
    ACTF = mybir.ActivationFunctionType

    nc = bacc.Bacc(None, target_bir_lowering=False)

    state = nc.declare_dram_parameter("state", [Bc, D], f32, isOutput=False)
    out = nc.declare_dram_parameter("out", [Bc, D], f32, isOutput=True)

    shifts = [0] if topk == 1 else [-1, 0, 1]
    tags = {0: "", -1: "m", 1: "p"}
    cshape = {
        "c_iota": ([1, 1, E], f32), "c_ltmask": ([1, NCOL, NCOL], f32),
        "c_uones": ([P, P], f32), "c_id32": ([P, P], f32), "c_id16": ([P, P], f16),
        "c_onesbd": ([P, P], f32),
    }
    for j in shifts:
        t = tags[j]
        cshape[f"c_wA{t}"] = ([P, NSG, P], f16)
        cshape[f"c_wB{t}"] = ([P, NSG, 4, P], f16)
        cshape[f"c_b1s{t}"] = ([P, NSG, 4, 1], f32)
        cshape[f"c_wC{t}"] = ([P, NSG, 4, 32], f16)
        cshape[f"c_b2s{t}"] = ([P, NSG, 1], f32)
    cparams = {n: nc.declare_dram_parameter(n, list(sh), dt, isOutput=False)
               for n, (sh, dt) in cshape.items()}

    if debug:
        XAB = nc.declare_dram_parameter("XAB", [NROW, 2 * D], f32, isOutput=True)
        Ys = {tags[j]: nc.declare_dram_parameter(f"Y{tags[j]}", [NROW, 128], f16, isOutput=True)
              for j in shifts}
        DSTD = nc.declare_dram_parameter("DSTD", [P, NCOL], i32, isOutput=True)
    else:
        XAB = nc.dram_tensor("XAB", [NROW, 2 * D], f32)  # sorted [state | xn] rows
        Ys = {tags[j]: nc.dram_tensor(f"Y{tags[j]}", [NROW, 128], f16) for j in shifts}
        DSTD = nc.dram_tensor("DSTD", [P, NCOL], i32)  # debug: dst indices
    dsti_d = nc.dram_tensor("dsti_d", [P, NCOL], mybir.dt.int16)

    from contextlib import ExitStack
    with tile.TileContext(nc) as tc, ExitStack() as ctx:
        cpool = ctx.enter_context(tc.tile_pool(name="consts", bufs=1))
        ppool = ctx.enter_context(tc.tile_pool(name="p1", bufs=1))
        pspool = ctx.enter_context(tc.tile_pool(name="ps1", bufs=1, space="PSUM"))
        gpool = ctx.enter_context(tc.tile_pool(name="p2", bufs=2))
        gps = ctx.enter_context(tc.tile_pool(name="ps2", bufs=1, space="PSUM"))

        # ---- constants into SBUF ----
        ct = {}
        for n, (sh, dt) in cshape.items():
            if sh[0] == 1:  # replicate across partitions for compute-engine reads
                rsh = [P] + list(sh[1:])
                t = cpool.tile(rsh, dt, tag=n)
                nc.sync.dma_start(out=t[:], in_=cparams[n][:].to_broadcast(rsh))
            else:
                t = cpool.tile(sh, dt, tag=n)
                nc.sync.dma_start(out=t[:], in_=cparams[n][:])
            ct[n] = t
        epsb = cpool.tile([P, 1], f32, tag="epsb")
        nc.vector.memset(epsb[:], EPS)

        # ---- zero-fill sorted buffer ----
        zb = cpool.tile([P, NROW * 2 * D // P], f32, tag="zb")
        nc.vector.memset(zb[:], 0.0)
        nc.sync.dma_start(out=XAB.rearrange("(p k) d -> p (k d)", p=P), in_=zb[:])

        # ---- phase 1: load, LN1, routing ----
        st = ppool.tile([P, NCOL, D], f32, tag="st")
        nc.sync.dma_start(out=st[:], in_=state.rearrange("(p n) d -> p n d", p=P))

        opv = st[:, :, OPCODE:OPCODE+1]                       # [P, NCOL, 1]
        # one-hot vs expert ids: eq39[p, n, e]
        eq39 = ppool.tile([P, NCOL, E], f32, tag="eq39")
        iota3 = ct["c_iota"][:].to_broadcast([P, NCOL, E])
        nc.vector.tensor_tensor(out=eq39[:], in0=opv.to_broadcast([P, NCOL, E]),
                                in1=iota3, op=OP.is_equal)
        # rowcnt[p, e] = sum_n eq39
        rowcnt = ppool.tile([P, E], f32, tag="rowcnt")
        nc.vector.tensor_reduce(out=rowcnt[:], in_=eq39[:].rearrange("p n e -> p e n"),
                                axis=AX, op=OP.add)
        # C1[p, e] = sum_{p'<p} rowcnt[p', e]
        pc1 = pspool.tile([P, E], f32, tag="pc1")
        nc.tensor.matmul(pc1[:], ct["c_uones"][:], rowcnt[:], start=True, stop=True)
        # comb[p, e] = 40*C1 + e
        comb = ppool.tile([P, 1, E], f32, tag="comb")
        nc.vector.tensor_scalar(out=comb[:, 0, :], in0=pc1[:], scalar1=float(NE),
                                scalar2=None, op0=OP.mult)
        nc.vector.tensor_tensor(out=comb[:, 0, :], in0=comb[:, 0, :],
                                in1=ct["c_iota"][:, 0, :].to_broadcast([P, E]), op=OP.add)
        # csel[p, n] = sum_e eq39 * comb
        msel = ppool.tile([P, NCOL, E], f32, tag="msel")
        nc.vector.tensor_tensor(out=msel[:], in0=eq39[:],
                                in1=comb[:].to_broadcast([P, NCOL, E]), op=OP.mult)
        csel = ppool.tile([P, NCOL], f32, tag="csel")
        nc.vector.tensor_reduce(out=csel[:], in_=msel[:], axis=AX, op=OP.add)
        # within-row rank c2[p, n] = #{n' < n same expert}
        eqp = ppool.tile([P, NCOL, NCOL], f32, tag="eqp")
        nc.vector.tensor_tensor(
            out=eqp[:], in0=opv.to_broadcast([P, NCOL, NCOL]),
            in1=opv.rearrange("p n d -> p d n").to_broadcast([P, NCOL, NCOL]),
            op=OP.is_equal)
        nc.vector.tensor_tensor(out=eqp[:], in0=eqp[:],
                                in1=ct["c_ltmask"][:].to_broadcast([P, NCOL, NCOL]),
                                op=OP.mult)
        c2 = ppool.tile([P, NCOL], f32, tag="c2")
        nc.vector.tensor_reduce(out=c2[:], in_=eqp[:], axis=AX, op=OP.add)
        # dst = csel + 40*c2   (fp32 exact), cast int32
        dstf = ppool.tile([P, NCOL], f32, tag="dstf")
        nc.vector.tensor_scalar(out=dstf[:], in0=c2[:], scalar1=float(NE),
                                scalar2=None, op0=OP.mult)
        nc.vector.tensor_tensor(out=dstf[:], in0=dstf[:], in1=csel[:], op=OP.add)
        dsti = ppool.tile([P, NCOL], i32, tag="dsti")
        nc.vector.tensor_copy(out=dsti[:], in_=dstf[:])
        nc.sync.dma_start(out=DSTD[:], in_=dsti[:])
        # wrapped+replicated idx buffer for the one-call dma_gather:
        # token j = n*128+p lives at [j%16 + 16k, j//16] for all 8 replicas k
        dsti16 = ppool.tile([P, NCOL], mybir.dt.int16, tag="dsti16")
        nc.vector.tensor_copy(out=dsti16[:], in_=dstf[:])
        nc.sync.dma_start(out=dsti_d[:], in_=dsti16[:])
        dstw = ppool.tile([P, NCOL * 8], mybir.dt.int16, tag="dstw")
        for k in range(8):
            nc.sync.dma_start(
                out=dstw[16*k:16*(k+1), :].rearrange("q (n r) -> q n r", r=8),
                in_=dsti_d.rearrange("(r q) n -> q n r", q=16))

        # ---- LN1 ----
        mt = ppool.tile([P, NCOL, 1], f32, tag="mt")
        nc.vector.tensor_reduce(out=mt[:, :, 0], in_=st[:], axis=AX, op=OP.add)
        nc.vector.tensor_scalar(out=mt[:, :, 0], in0=mt[:, :, 0], scalar1=1.0/D,
                                scalar2=None, op0=OP.mult)
        sqt = ppool.tile([P, NCOL, D], f32, tag="sqt")
        nc.scalar.activation(out=sqt[:], in_=st[:], func=ACTF.Square, scale=1.0)
        vt = ppool.tile([P, NCOL, 1], f32, tag="vt")
        nc.vector.tensor_reduce(out=vt[:, :, 0], in_=sqt[:], axis=AX, op=OP.add)
        nc.vector.tensor_scalar(out=vt[:, :, 0], in0=vt[:, :, 0], scalar1=1.0/D,
                                scalar2=None, op0=OP.mult)
        m2 = ppool.tile([P, NCOL, 1], f32, tag="m2")
        nc.vector.tensor_tensor(out=m2[:], in0=mt[:], in1=mt[:], op=OP.mult)
        nc.vector.tensor_tensor(out=vt[:], in0=vt[:], in1=m2[:], op=OP.subtract)
        rs1 = ppool.tile([P, NCOL, 1], f32, tag="rs1")
        nc.scalar.activation(out=rs1[:, :, 0], in_=vt[:, :, 0],
                             func=ACTF.Abs_reciprocal_sqrt, bias=epsb[:], scale=1.0)
        xnst = ppool.tile([P, NCOL, 2 * D], f32, tag="xnst")
        nc.vector.tensor_copy(out=xnst[:, :, 0:D], in_=st[:])
        nc.vector.tensor_tensor(out=xnst[:, :, D:2*D], in0=st[:],
                                in1=rs1[:].to_broadcast([P, NCOL, D]), op=OP.mult)
        # ---- scatter (HW indirect DMA supports [128,1] offsets only) ----
        for n in range(NCOL):
            nc.gpsimd.indirect_dma_start(
                out=XAB[:], out_offset=IndirectOffsetOnAxis(ap=dsti[:, n:n+1], axis=0),
                in_=xnst[:, n, :], in_offset=None)

        # ---- phase 2: supergroups ----
        H = PADSZ // 2  # 80 ranks per half
        run2 = stop_after not in ("route", "scatter")
        for j in (shifts if run2 else []):
            tg = tags[j]
            Y = Ys[tg]
            for s in range(NSG):
                # load + transpose to stacked8 [128, 160]
                xnS = gpool.tile([P, PADSZ], f16, tag="xnS")
                xbS = gpool.tile([P, PADSZ], f32, tag="xbS")
                XABv = XAB.rearrange("(c e) d -> c e d", e=NE)
                for h in range(2):
                    hA = gpool.tile([H, 8, D], f32, tag="hA")
                    nc.sync.dma_start(
                        out=hA[:],
                        in_=XABv[h*H:(h+1)*H, 8*s:8*s+8, D:2*D])
                    pt = gps.tile([P, H], f32, tag="ptA")
                    nc.tensor.transpose(pt[:], hA[:].rearrange("c e d -> c (e d)"),
                                        ct["c_id32"][0:H, 0:H])
                    nc.scalar.copy(out=xnS[:, h*H:(h+1)*H], in_=pt[:])
                    hB = gpool.tile([H, 8, D], f32, tag="hB")
                    nc.sync.dma_start(
                        out=hB[:],
                        in_=XABv[h*H:(h+1)*H, 8*s:8*s+8, 0:D])
                    ptb = gps.tile([P, H], f32, tag="ptB")
                    nc.tensor.transpose(ptb[:], hB[:].rearrange("c e d -> c (e d)"),
                                        ct["c_id32"][0:H, 0:H])
                    nc.scalar.copy(out=xbS[:, h*H:(h+1)*H], in_=ptb[:])

                # attn + x1
                psA = gps.tile([P, PADSZ], f32, tag="psA")
                nc.tensor.matmul(psA[:], ct[f"c_wA{tg}"][:, s, :], xnS[:], start=True, stop=True)
                x1sq = gpool.tile([P, 2 * PADSZ], f32, tag="x1sq")
                x1 = x1sq[:, 0:PADSZ]
                sq = x1sq[:, PADSZ:2*PADSZ]
                nc.vector.tensor_tensor(out=x1, in0=psA[:], in1=xbS[:], op=OP.add)
                nc.vector.tensor_tensor(out=sq, in0=x1, in1=x1, op=OP.mult)
                # stacked LN2 stats
                psS = gps.tile([P, 2 * PADSZ], f32, tag="psS")
                nc.tensor.matmul(psS[:], ct["c_onesbd"][:], x1sq[:], start=True, stop=True)
                msq = gpool.tile([P, PADSZ], f32, tag="msq")
                nc.scalar.activation(out=msq[:], in_=psS[:, 0:PADSZ],
                                     func=ACTF.Square, scale=1.0)
                vv = gpool.tile([P, PADSZ], f32, tag="vv")
                nc.vector.tensor_tensor(out=vv[:], in0=psS[:, PADSZ:2*PADSZ], in1=msq[:],
                                        op=OP.subtract)
                rstd = gpool.tile([P, PADSZ], f32, tag="rstd")
                nc.scalar.activation(out=rstd[:], in_=vv[:],
                                     func=ACTF.Abs_reciprocal_sqrt, bias=epsb[:],
                                     scale=1.0)
                xn2h = gpool.tile([P, PADSZ], f16, tag="xn2h")
                nc.vector.tensor_tensor(out=xn2h[:], in0=x1, in1=rstd[:], op=OP.mult)
                # x1 + b2 (residual base)
                x1pb = gpool.tile([P, PADSZ], f32, tag="x1pb")
                nc.vector.tensor_scalar(out=x1pb[:], in0=x1, scalar1=ct[f"c_b2s{tg}"][:, s, :],
                                        scalar2=None, op0=OP.add)
                # FFN
                yS = gpool.tile([P, PADSZ], f32, tag="yS")
                for i in range(4):
                    psB = gps.tile([P, PADSZ], f32, tag="psB")
                    nc.tensor.matmul(psB[:], ct[f"c_wB{tg}"][:, s, i, :], xn2h[:],
                                     start=True, stop=True)
                    hS = gpool.tile([P, PADSZ], f16, tag="hS")
                    nc.scalar.activation(out=hS[:], in_=psB[:], func=ACTF.Silu,
                                         bias=ct[f"c_b1s{tg}"][:, s, i, :], scale=1.0)
                    psC = gps.tile([32, PADSZ], f32, tag="psC")
                    nc.tensor.matmul(psC[:], ct[f"c_wC{tg}"][:, s, i, :], hS[:],
                                     start=True, stop=True)
                    nc.vector.tensor_tensor(out=yS[32*i:32*(i+1), :],
                                            in0=x1pb[32*i:32*(i+1), :], in1=psC[:],
                                            op=OP.add)
                # store back (transpose halves)
                for h in range(2):
                    pto = gps.tile([H, P], f32, tag="pto")
                    nc.tensor.transpose(pto[:], yS[:, h*H:(h+1)*H], ct["c_id32"][:, 0:P])
                    oT = gpool.tile([H, P], f16, tag="oT")
                    nc.scalar.copy(out=oT[:], in_=pto[:])
                    nc.sync.dma_start(
                        out=Y.rearrange("(c e) f -> c e f", e=NE)[h*H:(h+1)*H, 8*s:8*s+8, 0:D],
                        in_=oT[:].rearrange("c (e d) -> c e d", d=D))

        # ---- phase 3: gather + gates + store ----
        run3 = run2 and stop_after != "compute"
        if run3:
            _phase3(nc, tc, ppool, ct, Ys, dsti, dstw, st, topk, f32, f16, OP,
                    IndirectOffsetOnAxis, out)

    nc.finalize()
    return nc


def _phase3(nc, tc, ppool, ct, Ys, dsti, dstw, st, topk, f32, f16, OP,
            IndirectOffsetOnAxis, out):
        acc = ppool.tile([P, NCOL, D], f32, tag="acc")
        yg = ppool.tile([P, NCOL, 128], f16, tag="yg")
        nc.gpsimd.dma_gather(
            out_ap=yg[:], in_ap=Ys[""][:], idxs_ap=dstw[:],
            num_idxs=Bc, num_idxs_reg=Bc, elem_size=128, single_packet=False)
        nc.vector.tensor_scalar(out=acc[:], in0=yg[:, :, 0:D], scalar1=G0,
                                scalar2=None, op0=OP.mult)
        if topk == 3:
            # neighbor validity masks from opcode value
            for tg, cmpop, lim in (("m", OP.is_ge, 1.0), ("p", OP.is_le, float(E - 2))):
                ygn = ppool.tile([P, NCOL, D], f32, tag=f"yg{tg}")
                for n in range(NCOL):
                    nc.gpsimd.indirect_dma_start(
                        out=ygn[:, n, :], out_offset=None, in_=Ys[tg][:],
                        in_offset=IndirectOffsetOnAxis(ap=dsti[:, n:n+1], axis=0))
                msk = ppool.tile([P, NCOL, 1], f32, tag=f"msk{tg}")
                nc.vector.tensor_scalar(out=msk[:, :, 0], in0=st[:, :, OPCODE],
                                        scalar1=lim, scalar2=G1, op0=cmpop, op1=OP.mult)
                nc.vector.tensor_tensor(out=ygn[:], in0=ygn[:],
                                        in1=msk[:].to_broadcast([P, NCOL, D]), op=OP.mult)
                nc.vector.tensor_tensor(out=acc[:], in0=acc[:], in1=ygn[:], op=OP.add)
        nc.sync.dma_start(out=out.rearrange("(p n) d -> p n d", p=P), in_=acc[:])



_CACHE = {}


def _get_nc():
    key = ("nc", TOPK)
    if key not in _CACHE:
        _CACHE[key] = build_kernel(topk=TOPK)
    return _CACHE[key]


def kernel(state, Wq, Wk, Wv, Wo, W1, b1, W2, b2, **_unused):
    from concourse.bass_utils import run_bass_kernel_spmd

    state = np.ascontiguousarray(np.asarray(state, dtype=np.float32))
    consts = prep_consts(Wq, Wk, np.asarray(Wv, np.float32), np.asarray(Wo, np.float32),
                         np.asarray(W1, np.float32), np.asarray(b1, np.float32),
                         np.asarray(W2, np.float32), np.asarray(b2, np.float32),
                         topk=TOPK)
    nc = _get_nc()
    ncores = 8
    in_maps = []
    for c in range(ncores):
        m = {"state": state[c * Bc:(c + 1) * Bc]}
        m.update(consts)
        in_maps.append(m)
    res = run_bass_kernel_spmd(nc, in_maps, core_ids=list(range(ncores)))
    out = np.concatenate([res.results[c]["out"] for c in range(ncores)], axis=0)
    return out.astype(np.float32)


def profile_exec_time(inputs):
    """Run once with NTFF tracing and return max per-core HW exec time in ns."""
    import os
    import shutil
    from concourse.bass_utils import run_bass_kernel_spmd

    state = np.ascontiguousarray(np.asarray(inputs["state"], dtype=np.float32))
    consts = prep_consts(inputs["Wq"], inputs["Wk"], np.asarray(inputs["Wv"], np.float32),
                         np.asarray(inputs["Wo"], np.float32), np.asarray(inputs["W1"], np.float32),
                         np.asarray(inputs["b1"], np.float32), np.asarray(inputs["W2"], np.float32),
                         np.asarray(inputs["b2"], np.float32), topk=TOPK)
    nc = _get_nc()
    in_maps = []
    for c in range(8):
        m = {"state": state[c * Bc:(c + 1) * Bc]}
        m.update(consts)
        in_maps.append(m)
    tdir = "/root/problem/trace_out"
    shutil.rmtree(tdir, ignore_errors=True)
    os.makedirs(tdir, exist_ok=True)
    res = run_bass_kernel_spmd(nc, in_maps, core_ids=list(range(8)), trace=True,
                               tmpdir=tdir)
    return res.exec_time_ns


# revision 23
# speedup vs baseline: 1.1161x; 1.1161x over previous
"""Trainium2 Bass kernel for nn_C4StandardTransformer (MoE-routed transformer step).

kernel(**inputs) takes the FULL inputs (state [32768,16] + expert weights),
shards the batch across 8 NeuronCores (pure data parallel), runs an on-device
MoE-routed Bass kernel per core, and returns the full [32768,16] output.

Key algorithmic facts exploited:
 - The reference's attention softmax is over a length-1 axis, so w == 1 and
   Q/K/Wq/Wk are dead; attn = xn @ (Wo[e] @ Wv[e]).T.
 - The opcode slot holds exact integers, so the soft top-hat gates take only
   the constant values g0 = sigmoid(10)^2 (own expert), g1 ~ 4.54e-5
   (neighbors), g2 ~ 9.4e-14 (negligible). The kernel computes the top-1
   (own-expert) term exactly and optionally the two neighbor terms (TOPK=3).
 - Tokens are routed on device: counting-sort ranks are computed with DVE
   one-hot/prefix tricks plus one PE matmul against a strict-triangular
   constant, then tokens are scattered into a rank-major sorted buffer via
   indirect DMA, processed per 8-expert supergroup in an 8-token-stacked
   [128, 160] layout with block-diagonal fp16 matmuls, and gathered back.
"""
import sys
import numpy as np

for _p in ("/opt/trn_rl_repo", "/root/.axon_site/_ro/trn_rl_repo"):
    if _p not in sys.path:
        sys.path.append(_p)

TOPK = 1




E, D, DFF, OPCODE, EPS = 39, 16, 64, 6, 1e-5
Bc = 4096            # tokens per core
P = 128              # partitions
NCOL = Bc // P       # 32 free-dim token slots per partition
PADSZ = 160          # slots per expert per core
NE = 40              # padded expert count (8*5)
NSG = 5              # supergroups
NROW = PADSZ * NE    # sorted buffer rows
G0 = float(1.0 / (1.0 + np.exp(-10.0))) ** 2
G1 = float((1.0 / (1.0 + np.exp(-30.0))) * (1.0 / (1.0 + np.exp(10.0))))


def prep_consts(Wq, Wk, Wv, Wo, W1, b1, W2, b2, topk=1):
    """Host-side constant packing. Returns dict name -> np.ndarray.
    For topk=3, weight sets for expert shifts j in (-1, 0, +1) are packed
    (shift j means slot expert e is processed with weights of e+j; out-of
    range -> zeros)."""
    Wov = np.einsum('ejv,evd->ejd', Wo, Wv).astype(np.float32)  # attn = Wov @ xn
    # fold LayerNorm mean-subtraction into the contraction weights
    Wov = Wov - Wov.mean(axis=2, keepdims=True)
    W1 = (W1 - W1.mean(axis=2, keepdims=True)).astype(np.float32)

    def wslot(Warr, e, fill_shape):
        if 0 <= e < E:
            return Warr[e]
        return np.zeros(fill_shape, Warr.dtype)

    shifts = [0] if topk == 1 else [-1, 0, 1]
    consts = {}
    # [1, 39] expert iota and [1, 32, 32] strict-lower mask, [128,128] helpers
    consts["c_iota"] = np.arange(E, dtype=np.float32).reshape(1, 1, E)
    lt = np.tril(np.ones((NCOL, NCOL), np.float32), -1)  # mask[n, n'] = n' < n
    consts["c_ltmask"] = lt.reshape(1, NCOL, NCOL)
    # C1 matmul stationary: lhsT[k, m] = 1 if k < m (strict upper)
    consts["c_uones"] = np.triu(np.ones((P, P), np.float32), 1)
    consts["c_id32"] = np.eye(P, dtype=np.float32)
    consts["c_id16"] = np.eye(P, dtype=np.float16)
    # block-ones for stacked mean: out[t*16+j, c] = sum_d rhs[t*16+d, c]/16
    onesbd = np.zeros((P, P), np.float32)
    for t in range(8):
        onesbd[t*16:(t+1)*16, t*16:(t+1)*16] = 1.0 / 16.0
    consts["c_onesbd"] = onesbd
    # wrap stationaries: mm r maps dstf partition r*16+q to out rows q+16k (8 replicas)
    wrap = np.zeros((8, P, P), np.float32)
    for r in range(8):
        for q in range(16):
            for k in range(8):
                wrap[r, r*16 + q, q + 16*k] = 1.0
    consts["c_wrap"] = np.ascontiguousarray(wrap.transpose(1, 0, 2))

    for j in shifts:
        tag = {0: "", -1: "m", 1: "p"}[j]
        wA = np.zeros((NSG, P, P), np.float16)
        wB = np.zeros((NSG, 4, P, P), np.float16)
        b1s = np.zeros((NSG, 4, P, 1), np.float32)
        wC = np.zeros((NSG, 4, P, 32), np.float16)
        b2s = np.zeros((NSG, P, 1), np.float32)
        for s in range(NSG):
            for t in range(8):
                e = 8 * s + t + j
                wv = wslot(Wov, e, (D, D))
                # attn[d_out] = sum_d wv[d_out, d] xn[d]; lhsT[k=d, m=d_out]
                wA[s, t*16:(t+1)*16, t*16:(t+1)*16] = wv.T.astype(np.float16)
                b2s[s, t*16:(t+1)*16, 0] = wslot(b2, e, (D,))
            for i in range(4):
                for tt in range(2):
                    e = 8 * s + 2 * i + tt + j
                    w1 = wslot(W1, e, (DFF, D))     # h[f] = sum_d w1[f,d] xn2[d]
                    t = 2 * i + tt                   # input slot
                    wB[s, i, t*16:(t+1)*16, tt*64:(tt+1)*64] = w1.T.astype(np.float16)
                    b1s[s, i, tt*64:(tt+1)*64, 0] = wslot(b1, e, (DFF,))
                    w2 = wslot(W2, e, (D, DFF))     # ffn[d] = sum_f w2[d,f] h[f]
                    wC[s, i, tt*64:(tt+1)*64, tt*16:(tt+1)*16] = w2.T.astype(np.float16)
        consts[f"c_wA{tag}"] = np.ascontiguousarray(wA.transpose(1, 0, 2))
        consts[f"c_wB{tag}"] = np.ascontiguousarray(wB.transpose(2, 0, 1, 3))
        consts[f"c_b1s{tag}"] = np.ascontiguousarray(b1s.transpose(2, 0, 1, 3))
        consts[f"c_wC{tag}"] = np.ascontiguousarray(wC.transpose(2, 0, 1, 3))
        consts[f"c_b2s{tag}"] = np.ascontiguousarray(b2s.transpose(1, 0, 2))
    return consts


def build_kernel(topk=1, stop_after=None, debug=False):
    """Build the Bacc module. stop_after in (None, 'route', 'scatter',
    'compute') truncates for debugging (intermediates stay in dram tensors)."""
    import concourse.bass as bass
    import concourse.bacc as bacc
    import concourse.tile as tile
    from concourse import mybir
    from concourse.bass import IndirectOffsetOnAxis

    f32, f16, i32 = mybir.dt.float32, mybir.dt.float16, mybir.dt.int32
    AX = mybir.AxisListType.X
    OP = mybir.AluOpType
    ACTF = mybir.ActivationFunctionType

    nc = bacc.Bacc(None, target_bir_lowering=False)

    state = nc.declare_dram_parameter("state", [Bc, D], f32, isOutput=False)
    out = nc.declare_dram_parameter("out", [Bc, D], f32, isOutput=True)

    shifts = [0] if topk == 1 else [-1, 0, 1]
    tags = {0: "", -1: "m", 1: "p"}
    cshape = {
        "c_iota": ([1, 1, E], f32), "c_ltmask": ([1, NCOL, NCOL], f32),
        "c_uones": ([P, P], f32), "c_id32": ([P, P], f32), "c_id16": ([P, P], f16),
        "c_onesbd": ([P, P], f32),
        "c_wrap": ([P, 8, P], f32),
    }
    for j in shifts:
        t = tags[j]
        cshape[f"c_wA{t}"] = ([P, NSG, P], f16)
        cshape[f"c_wB{t}"] = ([P, NSG, 4, P], f16)
        cshape[f"c_b1s{t}"] = ([P, NSG, 4, 1], f32)
        cshape[f"c_wC{t}"] = ([P, NSG, 4, 32], f16)
        cshape[f"c_b2s{t}"] = ([P, NSG, 1], f32)
    cparams = {n: nc.declare_dram_parameter(n, list(sh), dt, isOutput=False)
               for n, (sh, dt) in cshape.items()}

    if debug:
        XAB = nc.declare_dram_parameter("XAB", [NROW, 2 * D], f32, isOutput=True)
        Ys = {tags[j]: nc.declare_dram_parameter(f"Y{tags[j]}", [NROW, 128], f16, isOutput=True)
              for j in shifts}
        DSTD = nc.declare_dram_parameter("DSTD", [P, NCOL], i32, isOutput=True)
    else:
        XAB = nc.dram_tensor("XAB", [NROW, 2 * D], f32)  # sorted [state | xn] rows
        Ys = {tags[j]: nc.dram_tensor(f"Y{tags[j]}", [NROW, 128], f16) for j in shifts}
        DSTD = nc.dram_tensor("DSTD", [P, NCOL], i32)  # debug: dst indices

    from contextlib import ExitStack
    with tile.TileContext(nc) as tc, ExitStack() as ctx:
        cpool = ctx.enter_context(tc.tile_pool(name="consts", bufs=1))
        ppool = ctx.enter_context(tc.tile_pool(name="p1", bufs=1))
        pspool = ctx.enter_context(tc.tile_pool(name="ps1", bufs=1, space="PSUM"))
        gpool = ctx.enter_context(tc.tile_pool(name="p2", bufs=2))
        gps = ctx.enter_context(tc.tile_pool(name="ps2", bufs=1, space="PSUM"))

        # ---- constants into SBUF ----
        ct = {}
        for n, (sh, dt) in cshape.items():
            if sh[0] == 1:  # replicate across partitions for compute-engine reads
                rsh = [P] + list(sh[1:])
                t = cpool.tile(rsh, dt, tag=n)
                nc.sync.dma_start(out=t[:], in_=cparams[n][:].to_broadcast(rsh))
            else:
                t = cpool.tile(sh, dt, tag=n)
                nc.sync.dma_start(out=t[:], in_=cparams[n][:])
            ct[n] = t
        epsb = cpool.tile([P, 1], f32, tag="epsb")
        nc.vector.memset(epsb[:], EPS)

        # ---- zero-fill sorted buffer ----
        zb = cpool.tile([P, NROW * 2 * D // P], f32, tag="zb")
        nc.vector.memset(zb[:], 0.0)
        nc.sync.dma_start(out=XAB.rearrange("(p k) d -> p (k d)", p=P), in_=zb[:])

        # ---- phase 1: load, LN1, routing ----
        st = ppool.tile([P, NCOL, D], f32, tag="st")
        nc.sync.dma_start(out=st[:], in_=state.rearrange("(p n) d -> p n d", p=P))

        opv = st[:, :, OPCODE:OPCODE+1]                       # [P, NCOL, 1]
        # one-hot vs expert ids: eq39[p, n, e]
        eq39 = ppool.tile([P, NCOL, E], f32, tag="eq39")
        iota3 = ct["c_iota"][:].to_broadcast([P, NCOL, E])
        nc.vector.tensor_tensor(out=eq39[:], in0=opv.to_broadcast([P, NCOL, E]),
                                in1=iota3, op=OP.is_equal)
        # rowcnt[p, e] = sum_n eq39
        rowcnt = ppool.tile([P, E], f32, tag="rowcnt")
        nc.vector.tensor_reduce(out=rowcnt[:], in_=eq39[:].rearrange("p n e -> p e n"),
                                axis=AX, op=OP.add)
        # C1[p, e] = sum_{p'<p} rowcnt[p', e]
        pc1 = pspool.tile([P, E], f32, tag="pc1")
        nc.tensor.matmul(pc1[:], ct["c_uones"][:], rowcnt[:], start=True, stop=True)
        # comb[p, e] = 40*C1 + e
        comb = ppool.tile([P, 1, E], f32, tag="comb")
        nc.vector.tensor_scalar(out=comb[:, 0, :], in0=pc1[:], scalar1=float(NE),
                                scalar2=None, op0=OP.mult)
        nc.vector.tensor_tensor(out=comb[:, 0, :], in0=comb[:, 0, :],
                                in1=ct["c_iota"][:, 0, :].to_broadcast([P, E]), op=OP.add)
        # csel[p, n] = sum_e eq39 * comb
        msel = ppool.tile([P, NCOL, E], f32, tag="msel")
        nc.vector.tensor_tensor(out=msel[:], in0=eq39[:],
                                in1=comb[:].to_broadcast([P, NCOL, E]), op=OP.mult)
        csel = ppool.tile([P, NCOL], f32, tag="csel")
        nc.vector.tensor_reduce(out=csel[:], in_=msel[:], axis=AX, op=OP.add)
        # within-row rank c2[p, n] = #{n' < n same expert}
        eqp = ppool.tile([P, NCOL, NCOL], f32, tag="eqp")
        nc.vector.tensor_tensor(
            out=eqp[:], in0=opv.to_broadcast([P, NCOL, NCOL]),
            in1=opv.rearrange("p n d -> p d n").to_broadcast([P, NCOL, NCOL]),
            op=OP.is_equal)
        nc.vector.tensor_tensor(out=eqp[:], in0=eqp[:],
                                in1=ct["c_ltmask"][:].to_broadcast([P, NCOL, NCOL]),
                                op=OP.mult)
        c2 = ppool.tile([P, NCOL], f32, tag="c2")
        nc.vector.tensor_reduce(out=c2[:], in_=eqp[:], axis=AX, op=OP.add)
        # dst = csel + 40*c2   (fp32 exact), cast int32
        dstf = ppool.tile([P, NCOL], f32, tag="dstf")
        nc.vector.tensor_scalar(out=dstf[:], in0=c2[:], scalar1=float(NE),
                                scalar2=None, op0=OP.mult)
        nc.vector.tensor_tensor(out=dstf[:], in0=dstf[:], in1=csel[:], op=OP.add)
        dsti = ppool.tile([P, NCOL], i32, tag="dsti")
        nc.vector.tensor_copy(out=dsti[:], in_=dstf[:])
        nc.sync.dma_start(out=DSTD[:], in_=dsti[:])
        # wrapped+replicated idx buffer for the one-call dma_gather, built on
        # chip: token j = n*128+p lives at [j%16 + 16k, j//16] for all replicas
        # k. mm r regroups partitions r*16+q into rows q (replicated 8x); the
        # int16 copy lands it at free offset r of each 8-wide slot.
        dstw = ppool.tile([P, NCOL, 8], mybir.dt.int16, tag="dstw")
        for r in range(8):
            psw = pspool.tile([P, NCOL], f32, tag="pc1")
            nc.tensor.matmul(psw[:], ct["c_wrap"][:, r, :], dstf[:],
                             start=True, stop=True)
            nc.vector.tensor_copy(out=dstw[:, :, r], in_=psw[:])

        # ---- LN1 ----
        mt = ppool.tile([P, NCOL, 1], f32, tag="mt")
        nc.vector.tensor_reduce(out=mt[:, :, 0], in_=st[:], axis=AX, op=OP.add)
        nc.vector.tensor_scalar(out=mt[:, :, 0], in0=mt[:, :, 0], scalar1=1.0/D,
                                scalar2=None, op0=OP.mult)
        sqt = ppool.tile([P, NCOL, D], f32, tag="sqt")
        nc.scalar.activation(out=sqt[:], in_=st[:], func=ACTF.Square, scale=1.0)
        vt = ppool.tile([P, NCOL, 1], f32, tag="vt")
        nc.vector.tensor_reduce(out=vt[:, :, 0], in_=sqt[:], axis=AX, op=OP.add)
        nc.vector.tensor_scalar(out=vt[:, :, 0], in0=vt[:, :, 0], scalar1=1.0/D,
                                scalar2=None, op0=OP.mult)
        m2 = ppool.tile([P, NCOL, 1], f32, tag="m2")
        nc.vector.tensor_tensor(out=m2[:], in0=mt[:], in1=mt[:], op=OP.mult)
        nc.vector.tensor_tensor(out=vt[:], in0=vt[:], in1=m2[:], op=OP.subtract)
        rs1 = ppool.tile([P, NCOL, 1], f32, tag="rs1")
        nc.scalar.activation(out=rs1[:, :, 0], in_=vt[:, :, 0],
                             func=ACTF.Abs_reciprocal_sqrt, bias=epsb[:], scale=1.0)
        xnst = ppool.tile([P, NCOL, 2 * D], f32, tag="xnst")
        nc.vector.tensor_copy(out=xnst[:, :, 0:D], in_=st[:])
        nc.vector.tensor_tensor(out=xnst[:, :, D:2*D], in0=st[:],
                                in1=rs1[:].to_broadcast([P, NCOL, D]), op=OP.mult)
        # ---- scatter (HW indirect DMA supports [128,1] offsets only) ----
        for n in range(NCOL):
            nc.gpsimd.indirect_dma_start(
                out=XAB[:], out_offset=IndirectOffsetOnAxis(ap=dsti[:, n:n+1], axis=0),
                in_=xnst[:, n, :], in_offset=None)

        # ---- phase 2: supergroups ----
        H = PADSZ // 2  # 80 ranks per half
        run2 = stop_after not in ("route", "scatter")
        for j in (shifts if run2 else []):
            tg = tags[j]
            Y = Ys[tg]
            for s in range(NSG):
                # load + transpose to stacked8 [128, 160]
                xnS = gpool.tile([P, PADSZ], f16, tag="xnS")
                xbS = gpool.tile([P, PADSZ], f32, tag="xbS")
                XABv = XAB.rearrange("(c e) d -> c e d", e=NE)
                for h in range(2):
                    hA = gpool.tile([H, 8, D], f32, tag="hA")
                    nc.sync.dma_start(
                        out=hA[:],
                        in_=XABv[h*H:(h+1)*H, 8*s:8*s+8, D:2*D])
                    pt = gps.tile([P, H], f32, tag="ptA")
                    nc.tensor.transpose(pt[:], hA[:].rearrange("c e d -> c (e d)"),
                                        ct["c_id32"][0:H, 0:H])
                    nc.scalar.copy(out=xnS[:, h*H:(h+1)*H], in_=pt[:])
                    hB = gpool.tile([H, 8, D], f32, tag="hB")
                    nc.sync.dma_start(
                        out=hB[:],
                        in_=XABv[h*H:(h+1)*H, 8*s:8*s+8, 0:D])
                    ptb = gps.tile([P, H], f32, tag="ptB")
                    nc.tensor.transpose(ptb[:], hB[:].rearrange("c e d -> c (e d)"),
                                        ct["c_id32"][0:H, 0:H])
                    nc.scalar.copy(out=xbS[:, h*H:(h+1)*H], in_=ptb[:])

                # attn + x1
                psA = gps.tile([P, PADSZ], f32, tag="psA")
                nc.tensor.matmul(psA[:], ct[f"c_wA{tg}"][:, s, :], xnS[:], start=True, stop=True)
                x1sq = gpool.tile([P, 2 * PADSZ], f32, tag="x1sq")
                x1 = x1sq[:, 0:PADSZ]
                sq = x1sq[:, PADSZ:2*PADSZ]
                nc.vector.tensor_tensor(out=x1, in0=psA[:], in1=xbS[:], op=OP.add)
                nc.vector.tensor_tensor(out=sq, in0=x1, in1=x1, op=OP.mult)
                # stacked LN2 stats
                psS = gps.tile([P, 2 * PADSZ], f32, tag="psS")
                nc.tensor.matmul(psS[:], ct["c_onesbd"][:], x1sq[:], start=True, stop=True)
                msq = gpool.tile([P, PADSZ], f32, tag="msq")
                nc.scalar.activation(out=msq[:], in_=psS[:, 0:PADSZ],
                                     func=ACTF.Square, scale=1.0)
                vv = gpool.tile([P, PADSZ], f32, tag="vv")
                nc.vector.tensor_tensor(out=vv[:], in0=psS[:, PADSZ:2*PADSZ], in1=msq[:],
                                        op=OP.subtract)
                rstd = gpool.tile([P, PADSZ], f32, tag="rstd")
                nc.scalar.activation(out=rstd[:], in_=vv[:],
                                     func=ACTF.Abs_reciprocal_sqrt, bias=epsb[:],
                                     scale=1.0)
                xn2h = gpool.tile([P, PADSZ], f16, tag="xn2h")
                nc.vector.tensor_tensor(out=xn2h[:], in0=x1, in1=rstd[:], op=OP.mult)
                # x1 + b2 (residual base)
                x1pb = gpool.tile([P, PADSZ], f32, tag="x1pb")
                nc.vector.tensor_scalar(out=x1pb[:], in0=x1, scalar1=ct[f"c_b2s{tg}"][:, s, :],
                                        scalar2=None, op0=OP.add)
                # FFN
                yS = gpool.tile([P, PADSZ], f32, tag="yS")
                for i in range(4):
                    psB = gps.tile([P, PADSZ], f32, tag="psB")
                    nc.tensor.matmul(psB[:], ct[f"c_wB{tg}"][:, s, i, :], xn2h[:],
                                     start=True, stop=True)
                    hS = gpool.tile([P, PADSZ], f16, tag="hS")
                    nc.scalar.activation(out=hS[:], in_=psB[:], func=ACTF.Silu,
                                         bias=ct[f"c_b1s{tg}"][:, s, i, :], scale=1.0)
                    psC = gps.tile([32, PADSZ], f32, tag="psC")
                    nc.tensor.matmul(psC[:], ct[f"c_wC{tg}"][:, s, i, :], hS[:],
                                     start=True, stop=True)
                    nc.vector.tensor_tensor(out=yS[32*i:32*(i+1), :],
                                            in0=x1pb[32*i:32*(i+1), :], in1=psC[:],
                                            op=OP.add)
                # store back (transpose halves)
                for h in range(2):
                    pto = gps.tile([H, P], f32, tag="pto")
                    nc.tensor.transpose(pto[:], yS[:, h*H:(h+1)*H], ct["c_id32"][:, 0:P])
                    oT = gpool.tile([H, P], f16, tag="oT")
                    nc.scalar.copy(out=oT[:], in_=pto[:])
                    nc.sync.dma_start(
                        out=Y.rearrange("(c e) f -> c e f", e=NE)[h*H:(h+1)*H, 8*s:8*s+8, 0:D],
                        in_=oT[:].rearrange("c (e d) -> c e d", d=D))

        # ---- phase 3: gather + gates + store ----
        run3 = run2 and stop_after != "compute"
        if run3:
            _phase3(nc, tc, ppool, ct, Ys, dsti, dstw, st, topk, f32, f16, OP,
                    IndirectOffsetOnAxis, out)

    nc.finalize()
    return nc


def _phase3(nc, tc, ppool, ct, Ys, dsti, dstw, st, topk, f32, f16, OP,
            IndirectOffsetOnAxis, out):
        acc = ppool.tile([P, NCOL, D], f32, tag="acc")
        yg = ppool.tile([P, NCOL, 128], f16, tag="yg")
        nc.gpsimd.dma_gather(
            out_ap=yg[:], in_ap=Ys[""][:],
            idxs_ap=dstw[:].rearrange("q n r -> q (n r)"),
            num_idxs=Bc, num_idxs_reg=Bc, elem_size=128, single_packet=False)
        nc.vector.tensor_scalar(out=acc[:], in0=yg[:, :, 0:D], scalar1=G0,
                                scalar2=None, op0=OP.mult)
        if topk == 3:
            # neighbor validity masks from opcode value
            for tg, cmpop, lim in (("m", OP.is_ge, 1.0), ("p", OP.is_le, float(E - 2))):
                ygn = ppool.tile([P, NCOL, D], f32, tag=f"yg{tg}")
                for n in range(NCOL):
                    nc.gpsimd.indirect_dma_start(
                        out=ygn[:, n, :], out_offset=None, in_=Ys[tg][:],
                        in_offset=IndirectOffsetOnAxis(ap=dsti[:, n:n+1], axis=0))
                msk = ppool.tile([P, NCOL, 1], f32, tag=f"msk{tg}")
                nc.vector.tensor_scalar(out=msk[:, :, 0], in0=st[:, :, OPCODE],
                                        scalar1=lim, scalar2=G1, op0=cmpop, op1=OP.mult)
                nc.vector.tensor_tensor(out=ygn[:], in0=ygn[:],
                                        in1=msk[:].to_broadcast([P, NCOL, D]), op=OP.mult)
                nc.vector.tensor_tensor(out=acc[:], in0=acc[:], in1=ygn[:], op=OP.add)
        nc.sync.dma_start(out=out.rearrange("(p n) d -> p n d", p=P), in_=acc[:])



_CACHE = {}


def _get_nc():
    key = ("nc", TOPK)
    if key not in _CACHE:
        _CACHE[key] = build_kernel(topk=TOPK)
    return _CACHE[key]


def kernel(state, Wq, Wk, Wv, Wo, W1, b1, W2, b2, **_unused):
    from concourse.bass_utils import run_bass_kernel_spmd

    state = np.ascontiguousarray(np.asarray(state, dtype=np.float32))
    consts = prep_consts(Wq, Wk, np.asarray(Wv, np.float32), np.asarray(Wo, np.float32),
                         np.asarray(W1, np.float32), np.asarray(b1, np.float32),
                         np.asarray(W2, np.float32), np.asarray(b2, np.float32),
                         topk=TOPK)
    nc = _get_nc()
    ncores = 8
    in_maps = []
    for c in range(ncores):
        m = {"state": state[c * Bc:(c + 1) * Bc]}
        m.update(consts)
        in_maps.append(m)
    res = run_bass_kernel_spmd(nc, in_maps, core_ids=list(range(ncores)))
    out = np.concatenate([res.results[c]["out"] for c in range(ncores)], axis=0)
    return out.astype(np.float32)


def profile_exec_time(inputs):
    """Run once with NTFF tracing and return max per-core HW exec time in ns."""
    import os
    import shutil
    from concourse.bass_utils import run_bass_kernel_spmd

    state = np.ascontiguousarray(np.asarray(inputs["state"], dtype=np.float32))
    consts = prep_consts(inputs["Wq"], inputs["Wk"], np.asarray(inputs["Wv"], np.float32),
                         np.asarray(inputs["Wo"], np.float32), np.asarray(inputs["W1"], np.float32),
                         np.asarray(inputs["b1"], np.float32), np.asarray(inputs["W2"], np.float32),
                         np.asarray(inputs["b2"], np.float32), topk=TOPK)
    nc = _get_nc()
    in_maps = []
    for c in range(8):
        m = {"state": state[c * Bc:(c + 1) * Bc]}
        m.update(consts)
        in_maps.append(m)
    tdir = "/root/problem/trace_out"
    shutil.rmtree(tdir, ignore_errors=True)
    os.makedirs(tdir, exist_ok=True)
    res = run_bass_kernel_spmd(nc, in_maps, core_ids=list(range(8)), trace=True,
                               tmpdir=tdir)
    return res.exec_time_ns
